# revision 8
# baseline (speedup 1.0000x reference)
"""Trainium2 Bass kernel for nn_NodeBlock (GNN message passing).

Computes, for a graph with N=100000 nodes and E=1600000 edges:
    agg = segment_sum(edge_attr, edge_index[1], N)        # [N, 64]
    h   = relu(concat([x, agg], 1) @ W1 + b1)             # [N, 256]
    out = h @ W2 + b2                                     # [N, 64]

Strategy (8 NeuronCores, no collectives):
  * Nodes are bin-packed into blocks of <= NODE_CAP=32 nodes such that each
    block receives <= T_MAX*128 = 512 edges (host-side serpentine packing on
    in-degree).  Blocks are assigned round-robin-free: core c owns blocks
    [c*400, (c+1)*400) == node slots [c*12800, (c+1)*12800).
  * Edges are bucketed by receiver block on the host, so each core's shard
    contains exactly the edges targeting its own nodes -> no all-reduce.
  * On device, each 128-edge tile is turned into a one-hot [128, 32] matrix
    (iota vs local-node-index compare) and matmul'd against the edge
    attributes, accumulating the per-block aggregate in PSUM:
        agg_T[64 feat, 32 nodes] += E_tile[128, 64].T @ onehot[128, 32]
  * The MLP runs on 256-node groups in feature-major layout (nodes on the
    free dim), so no transposes are needed anywhere on device.
"""

import sys

sys.path.insert(0, "/opt/trn_rl_repo")

import numpy as np

# ---------------------------------------------------------------- constants
N_NODES = 100000
N_EDGES = 1600000
D = 64            # d_node == d_edge == d_out
D_HID = 256
N_CORES = 8

NODE_CAP = 32         # node slots per block
N_BINS = 3200         # total blocks (divisible by N_CORES)
BLOCKS_PER_CORE = N_BINS // N_CORES          # 400
GROUP_BLOCKS = 8                             # blocks per MLP group
SLOTS = N_BINS * NODE_CAP                    # 102400 padded node slots
SLOTS_PER_CORE = SLOTS // N_CORES            # 12800

_cache = {}


# ---------------------------------------------------------------- packing
def _pack_nodes(deg):
    """Assign each node to a (block, slot) so that every block has at most
    NODE_CAP nodes and block in-degree sums are nearly equal (serpentine on
    sorted degree).  Returns (orig, inv): orig[slot] = node id or -1,
    inv[node] = slot."""
    order = np.argsort(-deg, kind="stable")
    n = order.size
    rows = np.arange(n) // N_BINS
    cols = np.arange(n) % N_BINS
    cols = np.where(rows % 2 == 0, cols, N_BINS - 1 - cols)
    slot = cols * NODE_CAP + rows
    orig = np.full(SLOTS, -1, dtype=np.int64)
    orig[slot] = order
    inv = np.empty(n, dtype=np.int64)
    inv[order] = slot
    return orig, inv


def _preprocess(x, edge_attr, receivers):
    """Build per-core device arrays.  Returns (in_maps, orig, t_max)."""
    deg = np.bincount(receivers, minlength=N_NODES)
    orig, inv = _pack_nodes(deg)

    eslot = inv[receivers]                  # node slot of each edge's receiver
    blk = eslot // NODE_CAP                 # block id per edge
    lidx = (eslot % NODE_CAP).astype(np.float32)

    counts = np.bincount(blk, minlength=N_BINS)
    t_max = max(4, int(-(-counts.max() // 128)))   # tiles (of 128 edges) per block
    cap = t_max * 128

    order_e = np.argsort(blk, kind="stable")
    blk_s = blk[order_e]
    starts = np.zeros(N_BINS, dtype=np.int64)
    np.cumsum(counts[:-1], out=starts[1:])
    pos = np.arange(N_EDGES) - starts[blk_s]       # position within block
    k = pos // 128
    p = pos % 128

    core = blk_s // BLOCKS_PER_CORE
    gblk = blk_s % BLOCKS_PER_CORE
    lidx_s = lidx[order_e]

    # padded node features, transposed per core
    xt_full = np.zeros((SLOTS, D), dtype=np.float32)
    valid = orig >= 0
    xt_full[valid] = x[orig[valid]]

    n_tiles = BLOCKS_PER_CORE * t_max
    in_maps = []
    for c in range(N_CORES):
        sel = core == c
        pay = np.zeros((BLOCKS_PER_CORE * 128, t_max * D), dtype=np.float32)
        pay.reshape(BLOCKS_PER_CORE * 128, t_max, D)[
            gblk[sel] * 128 + p[sel], k[sel], :
        ] = edge_attr[order_e[sel]]
        la = np.full((128, n_tiles), float(NODE_CAP), dtype=np.float32)
        la[p[sel], gblk[sel] * t_max + k[sel]] = lidx_s[sel]
        in_maps.append(
            {
                "edges": pay,
                "lidx": la,
                "xt": np.ascontiguousarray(
                    xt_full[c * SLOTS_PER_CORE : (c + 1) * SLOTS_PER_CORE].T
                ),
            }
        )
    return in_maps, orig, t_max


# ---------------------------------------------------------------- program
def _build_program(t_max, use_f32r):
    from contextlib import ExitStack

    import concourse.bacc as bacc
    import concourse.tile as tile
    from concourse import mybir

    f32 = mybir.dt.float32
    G = BLOCKS_PER_CORE // GROUP_BLOCKS      # MLP groups per core (50)
    TPG = GROUP_BLOCKS * t_max               # edge tiles per group
    NPG = GROUP_BLOCKS * NODE_CAP            # nodes per group (256)
    NT = BLOCKS_PER_CORE * t_max             # edge tiles per core
    S = SLOTS_PER_CORE

    nc = bacc.Bacc("TRN2", target_bir_lowering=False, debug=False)
    edges = nc.dram_tensor(
        "edges", [BLOCKS_PER_CORE * 128, t_max * D], f32, kind="ExternalInput"
    ).ap()
    lidx = nc.dram_tensor("lidx", [128, NT], f32, kind="ExternalInput").ap()
    xt = nc.dram_tensor("xt", [D, S], f32, kind="ExternalInput").ap()
    w1x = nc.dram_tensor("w1x", [D, D_HID], f32, kind="ExternalInput").ap()
    w1a = nc.dram_tensor("w1a", [D, D_HID], f32, kind="ExternalInput").ap()
    b1 = nc.dram_tensor("b1", [128, 2], f32, kind="ExternalInput").ap()
    w2 = nc.dram_tensor("w2", [128, 128], f32, kind="ExternalInput").ap()
    b2 = nc.dram_tensor("b2", [D, 1], f32, kind="ExternalInput").ap()
    out = nc.dram_tensor("out_t", [D, S], f32, kind="ExternalOutput").ap()

    mm_dt = mybir.dt.float32r if use_f32r else f32

    def r(ap):
        return ap.bitcast(mm_dt) if use_f32r else ap

    with tile.TileContext(nc) as tc, ExitStack() as ctx:
        const = ctx.enter_context(tc.tile_pool(name="const", bufs=1))
        epool = ctx.enter_context(tc.tile_pool(name="epool", bufs=3))
        ohpool = ctx.enter_context(tc.tile_pool(name="ohpool", bufs=3))
        apool = ctx.enter_context(tc.tile_pool(name="apool", bufs=3))
        hpool = ctx.enter_context(tc.tile_pool(name="hpool", bufs=4))
        ps_a = ctx.enter_context(tc.tile_pool(name="ps_a", bufs=2, space="PSUM"))
        ps_h = ctx.enter_context(tc.tile_pool(name="ps_h", bufs=4, space="PSUM"))
        ps_o = ctx.enter_context(tc.tile_pool(name="ps_o", bufs=2, space="PSUM"))

        iota_i = const.tile([128, TPG * NODE_CAP], mybir.dt.int32)
        nc.gpsimd.iota(
            iota_i[:], pattern=[[0, TPG], [1, NODE_CAP]], channel_multiplier=0
        )
        iota_f = const.tile([128, TPG * NODE_CAP], f32)
        nc.vector.tensor_copy(iota_f[:], iota_i[:])

        xt_sb = const.tile([D, S], f32)
        nc.sync.dma_start(xt_sb[:], xt[:])
        lidx_sb = const.tile([128, NT], f32)
        nc.sync.dma_start(lidx_sb[:], lidx[:])
        w1x_sb = const.tile([D, D_HID], f32)
        nc.sync.dma_start(w1x_sb[:], w1x[:])
        w1a_sb = const.tile([D, D_HID], f32)
        nc.sync.dma_start(w1a_sb[:], w1a[:])
        w2_sb = const.tile([128, 128], f32)
        nc.sync.dma_start(w2_sb[:], w2[:])
        b1_sb = const.tile([128, 2], f32)
        nc.sync.dma_start(b1_sb[:], b1[:])
        b2_sb = const.tile([D, 1], f32)
        nc.sync.dma_start(b2_sb[:], b2[:])
        out_sb = const.tile([D, S], f32)

        for g in range(G):
            ech = epool.tile([128, TPG * D], f32)
            src = edges[g * GROUP_BLOCKS * 128 : (g + 1) * GROUP_BLOCKS * 128, :]
            nc.sync.dma_start(
                ech[:].rearrange("p (b c) -> p b c", b=GROUP_BLOCKS),
                src.rearrange("(b p) c -> p b c", p=128),
            )

            oh = ohpool.tile([128, TPG * NODE_CAP], f32)
            nc.vector.tensor_tensor(
                out=oh[:].rearrange("p (t l) -> p t l", l=NODE_CAP),
                in0=iota_f[:].rearrange("p (t l) -> p t l", l=NODE_CAP),
                in1=lidx_sb[:, g * TPG : (g + 1) * TPG, None].to_broadcast(
                    [128, TPG, NODE_CAP]
                ),
                op=mybir.AluOpType.is_equal,
            )

            agg_ps = ps_a.tile([D, NPG], f32)
            for b in range(GROUP_BLOCKS):
                for kk in range(t_max):
                    t = b * t_max + kk
                    nc.tensor.matmul(
                        out=agg_ps[:, b * NODE_CAP : (b + 1) * NODE_CAP],
                        lhsT=ech[:, t * D : (t + 1) * D],
                        rhs=oh[:, t * NODE_CAP : (t + 1) * NODE_CAP],
                        start=(kk == 0),
                        stop=(kk == t_max - 1),
                    )
            agg_sb = apool.tile([D, NPG], f32)
            nc.vector.tensor_copy(agg_sb[:], agg_ps[:])

            xs = xt_sb[:, g * NPG : (g + 1) * NPG]
            hs = []
            for hh in range(2):
                h_ps = ps_h.tile([128, NPG], f32)
                nc.tensor.matmul(
                    out=h_ps[:],
                    lhsT=r(w1x_sb[:, hh * 128 : (hh + 1) * 128]),
                    rhs=r(xs),
                    start=True,
                    stop=False,
                )
                nc.tensor.matmul(
                    out=h_ps[:],
                    lhsT=r(w1a_sb[:, hh * 128 : (hh + 1) * 128]),
                    rhs=r(agg_sb[:]),
                    start=False,
                    stop=True,
                )
                h_sb = hpool.tile([128, NPG], f32)
                nc.scalar.activation(
                    h_sb[:],
                    h_ps[:],
                    mybir.ActivationFunctionType.Relu,
                    bias=b1_sb[:, hh : hh + 1],
                    scale=1.0,
                )
                hs.append(h_sb)

            o_ps = ps_o.tile([D, NPG], f32)
            nc.tensor.matmul(
                out=o_ps[:], lhsT=r(w2_sb[:, 0:D]), rhs=r(hs[0][:]),
                start=True, stop=False,
            )
            nc.tensor.matmul(
                out=o_ps[:], lhsT=r(w2_sb[:, D : 2 * D]), rhs=r(hs[1][:]),
                start=False, stop=True,
            )
            nc.vector.tensor_scalar(
                out=out_sb[:, g * NPG : (g + 1) * NPG],
                in0=o_ps[:],
                scalar1=b2_sb[:, 0:1],
                scalar2=None,
                op0=mybir.AluOpType.add,
            )

        nc.sync.dma_start(out[:], out_sb[:])

    nc.compile()
    return nc


def _get_program(t_max, use_f32r):
    key = (t_max, use_f32r)
    if key not in _cache:
        _cache[key] = _build_program(t_max, use_f32r)
    return _cache[key]


# ---------------------------------------------------------------- entry
def kernel(x, edge_attr, edge_index, pos, W1, b1, W2, b2, _trace=False, _tmpdir=None):
    from concourse.bass_utils import run_bass_kernel_spmd

    x = np.asarray(x, dtype=np.float32)
    edge_attr = np.asarray(edge_attr, dtype=np.float32)
    receivers = np.asarray(edge_index[1]).astype(np.int64)
    W1 = np.asarray(W1, dtype=np.float32)
    b1 = np.asarray(b1, dtype=np.float32)
    W2 = np.asarray(W2, dtype=np.float32)
    b2 = np.asarray(b2, dtype=np.float32)

    in_maps, orig, t_max = _preprocess(x, edge_attr, receivers)

    w1x_dev = np.ascontiguousarray(W1[0:D, :])                  # [64, 256]
    w1a_dev = np.ascontiguousarray(W1[D : 2 * D, :])            # [64, 256]
    b1_dev = np.ascontiguousarray(b1.reshape(2, 128).T)         # [128, 2]
    w2_dev = np.ascontiguousarray(
        np.concatenate([W2[0:128, :], W2[128:256, :]], axis=1)  # [128, 128]
    )
    b2_dev = np.ascontiguousarray(b2.reshape(D, 1))             # [64, 1]
    for m in in_maps:
        m["w1x"] = w1x_dev
        m["w1a"] = w1a_dev
        m["b1"] = b1_dev
        m["w2"] = w2_dev
        m["b2"] = b2_dev

    nc = _get_program(t_max, use_f32r=False)
    res = run_bass_kernel_spmd(
        nc, in_maps, list(range(N_CORES)), trace=_trace, tmpdir=_tmpdir
    )

    big = np.concatenate([r["out_t"] for r in res.results], axis=1)  # [64, SLOTS]
    valid = orig >= 0
    result = np.empty((N_NODES, D), dtype=np.float32)
    result[orig[valid]] = big.T[valid]
    if _trace:
        kernel.last_results = res
    return result
